# revision 1
# baseline (speedup 1.0000x reference)
"""Trainium2 Bass kernel for the e3nn-style InterModule:
   out = Linear2( NormAct( Linear1(x) ) )  over irreps
     IN  [(512,0),(256,1),(128,2)]  dim 1920
     MID [(1024,0),(512,1),(256,2)] dim 3840
     OUT = IN

Strategy (per core, data-parallel over N across 8 cores):
  - node blocks of 512; x loaded node-major, PE-transposed into a
    j-de-interleaved feature-major layout X^T (f32r)
  - Linear1: stationary W1 (f32r), moving X^T -> H^T in PSUM (fp32)
  - NormAct: nsq/sqrt on the sqrt ACT-set, sigmoid on the sigmoid set
    (2 table loads per block), g = h * sigmoid(||h||) stored f32r
  - Linear2: stationary G^T chunks, moving W2 -> node-major out in PSUM,
    interleave-assembled in SBUF, DMA'd out
  - All matmuls float32r (TF32-class, 1 cyc/row at free>=256)

Host side: shard x over 8 cores, prescale weights by 1/sqrt(mul_in).
"""

import math
from contextlib import ExitStack

import numpy as np

import concourse.bass as bass
import concourse.tile as tile
from concourse import bacc, mybir
from concourse.bass_utils import run_bass_kernel_spmd
from concourse.masks import make_identity

F32 = mybir.dt.float32
F32R = mybir.dt.float32r
AF = mybir.ActivationFunctionType
ALU = mybir.AluOpType

N_CORES = 8
N_TOTAL = 32768
N_CORE = N_TOTAL // N_CORES          # 4096
BLK = 512                            # nodes per block
NBLK = N_CORE // BLK                 # 8
NSUB = BLK // 128                    # 4

D_IN = 1920
D_OUT = 1920

# feature-tile order for X^T: l0 k0..3 | l1 (k,j) | l2 j0..4
FT_L0 = 0      # + k               (4 tiles)
FT_L1 = 4      # + k*3 + j         (6 tiles)
FT_L2 = 10     # + j               (5 tiles)
N_FT = 15


def _build():
    nc = bacc.Bacc(
        "TRN2", target_bir_lowering=False, debug=False, num_devices=N_CORES
    )

    x = nc.dram_tensor("x", [N_CORE, D_IN], F32, kind="ExternalInput").ap()
    w1_l0 = nc.dram_tensor("w1_l0", [512, 1024], F32, kind="ExternalInput").ap()
    w1_l1 = nc.dram_tensor("w1_l1", [256, 512], F32, kind="ExternalInput").ap()
    w1_l2 = nc.dram_tensor("w1_l2", [128, 256], F32, kind="ExternalInput").ap()
    w2_l0 = nc.dram_tensor("w2_l0", [1024, 512], F32, kind="ExternalInput").ap()
    w2_l1 = nc.dram_tensor("w2_l1", [512, 256], F32, kind="ExternalInput").ap()
    w2_l2 = nc.dram_tensor("w2_l2", [256, 128], F32, kind="ExternalInput").ap()
    out = nc.dram_tensor("out", [N_CORE, D_OUT], F32, kind="ExternalOutput").ap()

    with tile.TileContext(nc) as tc, ExitStack() as ctx:
        consts = ctx.enter_context(tc.tile_pool(name="consts", bufs=1))
        sb = ctx.enter_context(tc.tile_pool(name="sb", bufs=1))
        ps = ctx.enter_context(tc.tile_pool(name="ps", bufs=1, space="PSUM"))

        ident = consts.tile([128, 128], F32)
        make_identity(nc, ident)

        # ---- weights: DMA straight into f32r tiles (bitcast, PE rounds) ----
        w1r_l0 = consts.tile([128, 4, 1024], F32R)
        w1r_l1 = consts.tile([128, 2, 512], F32R)
        w1r_l2 = consts.tile([128, 256], F32R)
        w2r_l0 = consts.tile([128, 8, 512], F32R)
        w2r_l1 = consts.tile([128, 4, 256], F32R)
        w2r_l2 = consts.tile([128, 2, 128], F32R)
        nc.sync.dma_start(
            out=w1r_l0,
            in_=w1_l0.bitcast(F32R).rearrange("(t p) v -> p t v", p=128),
        )
        nc.sync.dma_start(
            out=w1r_l1,
            in_=w1_l1.bitcast(F32R).rearrange("(t p) v -> p t v", p=128),
        )
        nc.sync.dma_start(out=w1r_l2, in_=w1_l2.bitcast(F32R))
        nc.sync.dma_start(
            out=w2r_l0,
            in_=w2_l0.bitcast(F32R).rearrange("(t p) v -> p t v", p=128),
        )
        nc.sync.dma_start(
            out=w2r_l1,
            in_=w2_l1.bitcast(F32R).rearrange("(t p) v -> p t v", p=128),
        )
        nc.sync.dma_start(
            out=w2r_l2,
            in_=w2_l2.bitcast(F32R).rearrange("(t p) v -> p t v", p=128),
        )

        # ---- persistent per-block SBUF ----
        xt = sb.tile([128, N_FT, BLK], F32R, name="xt")        # X^T, de-interleaved
        g_l0 = sb.tile([128, 8, BLK], F32R, name="g_l0")
        g_l1 = sb.tile([128, 4, 3, BLK], F32R, name="g_l1")
        g_l2 = sb.tile([128, 2, 5, BLK], F32R, name="g_l2")
        nbuf = sb.tile([128, 6, BLK], F32, name="nbuf")        # norms (l1 x4, l2 x2)

        for b in range(NBLK):
            # ================= stage 1: load + transpose =================
            x_nat = sb.tile([128, NSUB, D_IN], F32, name="x_nat", tag="x_nat", bufs=1)
            nc.sync.dma_start(
                out=x_nat,
                in_=x[b * BLK : (b + 1) * BLK, :].rearrange(
                    "(s p) f -> p s f", p=128
                ),
            )
            def tr_src(ft, s):
                if ft < FT_L1:
                    k = ft
                    return x_nat[:, s, k * 128 : (k + 1) * 128]
                if ft < FT_L2:
                    k, j = divmod(ft - FT_L1, 3)
                    v = x_nat[:, s, 512:1280].rearrange("p (u j) -> p u j", j=3)
                    return v[:, k * 128 : (k + 1) * 128, j]
                j = ft - FT_L2
                v = x_nat[:, s, 1280:1920].rearrange("p (u j) -> p u j", j=5)
                return v[:, :, j]

            for grp in range(5):  # 5 groups x 3 feature tiles
                st = ps.tile([128, 3, BLK], F32, name="st", tag="hps", bufs=2)
                for c in range(3):
                    ft = grp * 3 + c
                    for s in range(NSUB):
                        nc.tensor.transpose(
                            st[:, c, s * 128 : (s + 1) * 128], tr_src(ft, s), ident
                        )
                nc.vector.tensor_copy(
                    out=xt[:, grp * 3 : (grp + 1) * 3, :], in_=st
                )

            # ============ phase A (sqrt set): l1 + l2 of Linear1 ============
            # l1: mid muls 512 -> kv 0..3, contraction 256 -> ki 0..1
            for kv in range(4):
                hm = ps.tile([128, 3, BLK], F32, name="hm", tag="hps", bufs=2)
                for j in range(3):
                    for ki in range(2):
                        nc.tensor.matmul(
                            hm[:, j, :],
                            w1r_l1[:, ki, kv * 128 : (kv + 1) * 128],
                            xt[:, FT_L1 + ki * 3 + j, :],
                            start=(ki == 0),
                            stop=(ki == 1),
                        )
                nc.scalar.activation(out=g_l1[:, kv], in_=hm, func=AF.Copy)
                sq = sb.tile([128, 3, BLK], F32, name="sq", tag="sq", bufs=2)
                nc.vector.tensor_mul(sq, hm, g_l1[:, kv])
                nsq = nbuf[:, kv, :]
                nc.vector.tensor_add(nsq, sq[:, 0, :], sq[:, 1, :])
                nc.vector.tensor_add(nsq, nsq, sq[:, 2, :])
                nc.scalar.activation(out=nsq, in_=nsq, func=AF.Sqrt)
            # l2: mid muls 256 -> kv 0..1, contraction 128 (single ki), 5 j's
            for kv in range(2):
                hm1 = ps.tile([128, 3, BLK], F32, name="hm1", tag="hps", bufs=2)
                for j in range(3):
                    nc.tensor.matmul(
                        hm1[:, j, :],
                        w1r_l2[:, kv * 128 : (kv + 1) * 128],
                        xt[:, FT_L2 + j, :],
                        start=True,
                        stop=True,
                    )
                hm2 = ps.tile([128, 3, BLK], F32, name="hm2", tag="hps", bufs=2)
                for j in range(3, 5):
                    nc.tensor.matmul(
                        hm2[:, j - 3, :],
                        w1r_l2[:, kv * 128 : (kv + 1) * 128],
                        xt[:, FT_L2 + j, :],
                        start=True,
                        stop=True,
                    )
                nc.scalar.activation(out=g_l2[:, kv, 0:3, :], in_=hm1, func=AF.Copy)
                nc.scalar.activation(
                    out=g_l2[:, kv, 3:5, :], in_=hm2[:, 0:2, :], func=AF.Copy
                )
                sq1 = sb.tile([128, 3, BLK], F32, name="sq1", tag="sq", bufs=2)
                nc.vector.tensor_mul(sq1, hm1, g_l2[:, kv, 0:3, :])
                sq2 = sb.tile([128, 3, BLK], F32, name="sq2", tag="sq", bufs=2)
                nc.vector.tensor_mul(
                    sq2[:, 0:2, :], hm2[:, 0:2, :], g_l2[:, kv, 3:5, :]
                )
                nsq = nbuf[:, 4 + kv, :]
                nc.vector.tensor_add(nsq, sq1[:, 0, :], sq1[:, 1, :])
                nc.vector.tensor_add(nsq, nsq, sq1[:, 2, :])
                nc.vector.tensor_add(nsq, nsq, sq2[:, 0, :])
                nc.vector.tensor_add(nsq, nsq, sq2[:, 1, :])
                nc.scalar.activation(out=nsq, in_=nsq, func=AF.Sqrt)

            # ========= phase B (sigmoid set): l0 of Linear1 + scales =========
            for kv in range(8):
                h1 = ps.tile([128, BLK], F32, name="h1", tag="ps1", bufs=2)
                for ki in range(4):
                    nc.tensor.matmul(
                        h1,
                        w1r_l0[:, ki, kv * 128 : (kv + 1) * 128],
                        xt[:, FT_L0 + ki, :],
                        start=(ki == 0),
                        stop=(ki == 3),
                    )
                n0 = sb.tile([128, BLK], F32, name="n0", tag="s", bufs=3)
                nc.scalar.activation(out=n0, in_=h1, func=AF.Abs)
                nc.scalar.activation(out=n0, in_=n0, func=AF.Sigmoid)
                nc.vector.tensor_mul(g_l0[:, kv, :], h1, n0)
            for kv in range(4):
                sl = sb.tile([128, BLK], F32, name="sl", tag="s", bufs=3)
                nc.scalar.activation(out=sl, in_=nbuf[:, kv, :], func=AF.Sigmoid)
                nc.vector.tensor_mul(
                    g_l1[:, kv],
                    g_l1[:, kv],
                    sl.unsqueeze(1).broadcast_to([128, 3, BLK]),
                )
            for kv in range(2):
                sl2 = sb.tile([128, BLK], F32, name="sl2", tag="s", bufs=3)
                nc.scalar.activation(out=sl2, in_=nbuf[:, 4 + kv, :], func=AF.Sigmoid)
                nc.vector.tensor_mul(
                    g_l2[:, kv],
                    g_l2[:, kv],
                    sl2.unsqueeze(1).broadcast_to([128, 5, BLK]),
                )

            # ================= stage 3: Linear2 (l1, l2, l0) =================
            for ns in range(NSUB):
                nsl = slice(ns * 128, (ns + 1) * 128)
                outsb = sb.tile([128, D_OUT], F32, name="outsb", tag="outsb", bufs=2)
                ov1 = outsb[:, 512:1280].rearrange("p (v j) -> p v j", j=3)
                ov2 = outsb[:, 1280:1920].rearrange("p (v j) -> p v j", j=5)
                # l1: out muls 256, contraction 512 -> ku 0..3
                q1 = ps.tile([128, 2, 256], F32, name="q1", tag="ps1", bufs=2)
                for idx, j in enumerate((0, 1)):
                    for ku in range(4):
                        nc.tensor.matmul(
                            q1[:, idx, :],
                            g_l1[:, ku, j, nsl],
                            w2r_l1[:, ku, :],
                            start=(ku == 0),
                            stop=(ku == 3),
                        )
                nc.scalar.activation(
                    out=ov1[:, :, 0:2], in_=q1.rearrange("p j v -> p v j"), func=AF.Copy
                )
                q2 = ps.tile([128, 2, 256], F32, name="q2", tag="ps1", bufs=2)
                for ku in range(4):
                    nc.tensor.matmul(
                        q2[:, 0, :],
                        g_l1[:, ku, 2, nsl],
                        w2r_l1[:, ku, :],
                        start=(ku == 0),
                        stop=(ku == 3),
                    )
                nc.scalar.activation(
                    out=ov1[:, :, 2:3],
                    in_=q2[:, 0:1, :].rearrange("p j v -> p v j"),
                    func=AF.Copy,
                )
                # l2: out muls 128, contraction 256 -> ku 0..1
                q3 = ps.tile([128, 4, 128], F32, name="q3", tag="ps1", bufs=2)
                for j in range(4):
                    for ku in range(2):
                        nc.tensor.matmul(
                            q3[:, j, :],
                            g_l2[:, ku, j, nsl],
                            w2r_l2[:, ku, :],
                            start=(ku == 0),
                            stop=(ku == 1),
                        )
                nc.scalar.activation(
                    out=ov2[:, :, 0:4], in_=q3.rearrange("p j v -> p v j"), func=AF.Copy
                )
                q4 = ps.tile([128, 4, 128], F32, name="q4", tag="ps1", bufs=2)
                for ku in range(2):
                    nc.tensor.matmul(
                        q4[:, 0, :],
                        g_l2[:, ku, 4, nsl],
                        w2r_l2[:, ku, :],
                        start=(ku == 0),
                        stop=(ku == 1),
                    )
                nc.scalar.activation(
                    out=ov2[:, :, 4:5],
                    in_=q4[:, 0:1, :].rearrange("p j v -> p v j"),
                    func=AF.Copy,
                )
                # l0: out muls 512, contraction 1024 -> ku 0..7
                q0 = ps.tile([128, 512], F32, name="q0", tag="ps1", bufs=2)
                for ku in range(8):
                    nc.tensor.matmul(
                        q0,
                        g_l0[:, ku, nsl],
                        w2r_l0[:, ku, :],
                        start=(ku == 0),
                        stop=(ku == 7),
                    )
                nc.scalar.activation(out=outsb[:, 0:512], in_=q0, func=AF.Copy)

                nc.sync.dma_start(
                    out=out[b * BLK + ns * 128 : b * BLK + (ns + 1) * 128, :],
                    in_=outsb,
                )

    nc.compile()
    return nc


_NC_CACHE = None


def _get_nc():
    global _NC_CACHE
    if _NC_CACHE is None:
        _NC_CACHE = _build()
    return _NC_CACHE


def kernel(x, w1_l0, w1_l1, w1_l2, w2_l0, w2_l1, w2_l2):
    x = np.ascontiguousarray(np.asarray(x, dtype=np.float32))
    ws = {
        "w1_l0": np.ascontiguousarray(
            np.asarray(w1_l0, np.float32) / math.sqrt(512.0)
        ),
        "w1_l1": np.ascontiguousarray(
            np.asarray(w1_l1, np.float32) / math.sqrt(256.0)
        ),
        "w1_l2": np.ascontiguousarray(
            np.asarray(w1_l2, np.float32) / math.sqrt(128.0)
        ),
        "w2_l0": np.ascontiguousarray(
            np.asarray(w2_l0, np.float32) / math.sqrt(1024.0)
        ),
        "w2_l1": np.ascontiguousarray(
            np.asarray(w2_l1, np.float32) / math.sqrt(512.0)
        ),
        "w2_l2": np.ascontiguousarray(
            np.asarray(w2_l2, np.float32) / math.sqrt(256.0)
        ),
    }
    nc = _get_nc()
    in_maps = [
        {"x": x[c * N_CORE : (c + 1) * N_CORE], **ws} for c in range(N_CORES)
    ]
    res = run_bass_kernel_spmd(nc, in_maps, list(range(N_CORES))).results
    return np.concatenate([res[c]["out"] for c in range(N_CORES)], axis=0)



# revision 3
# speedup vs baseline: 1.7912x; 1.7912x over previous
"""Trainium2 Bass kernel for the e3nn-style InterModule:
   out = Linear2( NormAct( Linear1(x) ) )  over irreps
     IN  [(512,0),(256,1),(128,2)]  dim 1920
     MID [(1024,0),(512,1),(256,2)] dim 3840
     OUT = IN
   N = 32768 nodes, data-parallel over 8 cores (4096 nodes/core).

v2 design (feature-major end-to-end, bf16):
  - Host pre-transposes x into de-interleaved feature-major tiles
    xt[128p, 15ft, 4096n] (bf16) and pre-transposes/prescales weights,
    so the device does NO transposes at all.
  - Linear1: stationary W1 tile [u,128v], moving xt [u, n] -> h^T in PSUM.
  - NormAct: h copied PSUM->SBUF bf16 (ACT/DVE split), squares + adds on
    DVE bf16 2x, one batched Sqrt + Sigmoids per block (2 ACT table
    loads/block), scale-mul in place.
  - Linear2: stationary W2 tile [v,128w], moving g -> out^T in PSUM,
    copied to SBUF bf16, DMA'd out feature-major; host re-interleaves
    and upcasts to f32.

hsb row map (mid irreps): l0: rows 0..8 (kv), l1: rows 8..20 (kv*3+j),
l2: rows 20..30 (kv*5+j).  out^T row map: l0: 0..4 (wt), l1: 4..10
(ko*3+j), l2: 10..15 (j).
"""

import math
from contextlib import ExitStack

import numpy as np
import ml_dtypes

import concourse.bass as bass
import concourse.tile as tile
from concourse import bacc, mybir
from concourse.bass_utils import run_bass_kernel_spmd

F32 = mybir.dt.float32
BF16 = mybir.dt.bfloat16
AF = mybir.ActivationFunctionType
ALU = mybir.AluOpType

BF = ml_dtypes.bfloat16

N_CORES = 8
N_TOTAL = 32768
N_CORE = N_TOTAL // N_CORES          # 4096
BLK = 512
NBLK = N_CORE // BLK                 # 8

# L1 copy tiles routed to DVE instead of ACT (of 10 [3,512] tiles)
L1_DVE_TILES = {5}
# stage-3 out slices routed to DVE instead of ACT (of 15)
S3_DVE = {0, 1, 2, 3, 4, 5, 6, 7, 8}


def _build():
    nc = bacc.Bacc(
        "TRN2", target_bir_lowering=False, debug=False, num_devices=N_CORES
    )

    xt_d = nc.dram_tensor("xt", [128, 15, N_CORE], BF16, kind="ExternalInput").ap()
    w1l0_d = nc.dram_tensor("w1l0", [128, 4, 1024], BF16, kind="ExternalInput").ap()
    w1l1_d = nc.dram_tensor("w1l1", [128, 2, 512], BF16, kind="ExternalInput").ap()
    w1l2_d = nc.dram_tensor("w1l2", [128, 256], BF16, kind="ExternalInput").ap()
    w2l0_d = nc.dram_tensor("w2l0", [128, 8, 512], BF16, kind="ExternalInput").ap()
    w2l1_d = nc.dram_tensor("w2l1", [128, 4, 256], BF16, kind="ExternalInput").ap()
    w2l2_d = nc.dram_tensor("w2l2", [128, 2, 128], BF16, kind="ExternalInput").ap()
    out_d = nc.dram_tensor("out", [128, 15, N_CORE], BF16, kind="ExternalOutput").ap()

    with tile.TileContext(nc) as tc, ExitStack() as ctx:
        consts = ctx.enter_context(tc.tile_pool(name="consts", bufs=1))
        sb = ctx.enter_context(tc.tile_pool(name="sb", bufs=1))
        ps = ctx.enter_context(tc.tile_pool(name="ps", bufs=1, space="PSUM"))

        w1l0 = consts.tile([128, 4, 1024], BF16)
        w1l1 = consts.tile([128, 2, 512], BF16)
        w1l2 = consts.tile([128, 256], BF16)
        w2l0 = consts.tile([128, 8, 512], BF16)
        w2l1 = consts.tile([128, 4, 256], BF16)
        w2l2 = consts.tile([128, 2, 128], BF16)
        nc.sync.dma_start(out=w1l0, in_=w1l0_d)
        nc.sync.dma_start(out=w1l1, in_=w1l1_d)
        nc.sync.dma_start(out=w1l2, in_=w1l2_d)
        nc.sync.dma_start(out=w2l0, in_=w2l0_d)
        nc.sync.dma_start(out=w2l1, in_=w2l1_d)
        nc.sync.dma_start(out=w2l2, in_=w2l2_d)

        def l1_slice(s, xt):
            """Accumulation list for mid slice s: [(stationary, moving), ...]"""
            if s < 8:
                kv = s
                return [
                    (w1l0[:, ki, kv * 128 : (kv + 1) * 128], xt[:, ki, :])
                    for ki in range(4)
                ]
            if s < 20:
                kv, j = divmod(s - 8, 3)
                return [
                    (w1l1[:, ki, kv * 128 : (kv + 1) * 128], xt[:, 4 + ki * 3 + j, :])
                    for ki in range(2)
                ]
            kv, j = divmod(s - 20, 5)
            return [(w1l2[:, kv * 128 : (kv + 1) * 128], xt[:, 10 + j, :])]

        def s3_slice(ft, hsb):
            if ft < 4:
                wt = ft
                return [
                    (w2l0[:, kv, wt * 128 : (wt + 1) * 128], hsb[:, kv, :])
                    for kv in range(8)
                ]
            if ft < 10:
                ko, j = divmod(ft - 4, 3)
                return [
                    (w2l1[:, kv, ko * 128 : (ko + 1) * 128], hsb[:, 8 + kv * 3 + j, :])
                    for kv in range(4)
                ]
            j = ft - 10
            return [(w2l2[:, kv, :], hsb[:, 20 + kv * 5 + j, :]) for kv in range(2)]

        for b in range(NBLK):
            xt = sb.tile([128, 15, BLK], BF16, name="xt", tag="xt", bufs=2)
            nc.sync.dma_start(out=xt, in_=xt_d[:, :, b * BLK : (b + 1) * BLK])

            hsb = sb.tile([128, 30, BLK], BF16, name="hsb", tag="hsb", bufs=2)
            sqb = sb.tile([128, 22, BLK], BF16, name="sqb", tag="sqb", bufs=1)
            nrm = sb.tile([128, 6, BLK], BF16, name="nrm", tag="nrm", bufs=2)
            absb = sb.tile([128, 8, BLK], BF16, name="absb", tag="absb", bufs=2)
            outsb = sb.tile([128, 15, BLK], BF16, name="outsb", tag="outsb", bufs=2)

            # ---------------- Linear1: 30 mid slices via [3,512] PSUM tiles
            for t in range(10):
                hm = ps.tile([128, 3, BLK], F32, name="hm", tag="hA", bufs=2)
                for r in range(3):
                    mms = l1_slice(t * 3 + r, xt)
                    for k, (w_ap, x_ap) in enumerate(mms):
                        nc.tensor.matmul(
                            hm[:, r, :], w_ap, x_ap,
                            start=(k == 0), stop=(k == len(mms) - 1),
                        )
                dst = hsb[:, t * 3 : (t + 1) * 3, :]
                if t in L1_DVE_TILES:
                    nc.vector.tensor_copy(out=dst, in_=hm)
                else:
                    nc.scalar.activation(out=dst, in_=hm, func=AF.Copy)

            # ---------------- NormAct
            # squares of l1/l2 rows (hsb 8..30 -> sqb 0..22), bf16 2x
            for (a, e) in ((8, 14), (14, 20), (20, 26), (26, 30)):
                nc.vector.tensor_mul(
                    sqb[:, a - 8 : e - 8, :], hsb[:, a:e, :], hsb[:, a:e, :]
                )
            # nsq: l1 (j=3) -> nrm[0:4], l2 (j=5) -> nrm[4:6]
            v3 = sqb[:, 0:12, :].rearrange("p (k j) n -> p k j n", j=3)
            nc.vector.tensor_add(nrm[:, 0:4, :], v3[:, :, 0, :], v3[:, :, 1, :])
            nc.vector.tensor_add(nrm[:, 0:4, :], nrm[:, 0:4, :], v3[:, :, 2, :])
            v5 = sqb[:, 12:22, :].rearrange("p (k j) n -> p k j n", j=5)
            nc.vector.tensor_add(nrm[:, 4:6, :], v5[:, :, 0, :], v5[:, :, 1, :])
            for j in (2, 3, 4):
                nc.vector.tensor_add(nrm[:, 4:6, :], nrm[:, 4:6, :], v5[:, :, j, :])
            # |h| for l0: max(-h, h)
            nc.vector.scalar_tensor_tensor(
                out=absb, in0=hsb[:, 0:8, :], scalar=-1.0, in1=hsb[:, 0:8, :],
                op0=ALU.mult, op1=ALU.max,
            )
            # n = sqrt(nsq)   (sqrt table set)
            nc.scalar.activation(out=nrm, in_=nrm, func=AF.Sqrt)
            # scales = sigmoid(n)  (sigmoid table set)
            nc.scalar.activation(out=nrm, in_=nrm, func=AF.Sigmoid)
            nc.scalar.activation(out=absb, in_=absb, func=AF.Sigmoid)
            # g = h * scale (in place on hsb)
            nc.vector.tensor_mul(hsb[:, 0:8, :], hsb[:, 0:8, :], absb)
            h3 = hsb[:, 8:20, :].rearrange("p (k j) n -> p k j n", j=3)
            nc.vector.tensor_mul(
                h3, h3,
                nrm[:, 0:4, :].unsqueeze(2).broadcast_to([128, 4, 3, BLK]),
            )
            h5 = hsb[:, 20:30, :].rearrange("p (k j) n -> p k j n", j=5)
            nc.vector.tensor_mul(
                h5, h5,
                nrm[:, 4:6, :].unsqueeze(2).broadcast_to([128, 2, 5, BLK]),
            )

            # ---------------- Linear2: 15 out slices via [512] PSUM tiles
            for ft in range(15):
                q = ps.tile([128, BLK], F32, name="q", tag="hB", bufs=2)
                mms = s3_slice(ft, hsb)
                for k, (w_ap, g_ap) in enumerate(mms):
                    nc.tensor.matmul(
                        q, w_ap, g_ap, start=(k == 0), stop=(k == len(mms) - 1)
                    )
                if ft in S3_DVE:
                    nc.vector.tensor_copy(out=outsb[:, ft, :], in_=q)
                else:
                    nc.scalar.activation(out=outsb[:, ft, :], in_=q, func=AF.Copy)

            nc.sync.dma_start(
                out=out_d[:, :, b * BLK : (b + 1) * BLK], in_=outsb
            )

    nc.compile()
    return nc


_NC_CACHE = None


def _get_nc():
    global _NC_CACHE
    if _NC_CACHE is None:
        _NC_CACHE = _build()
    return _NC_CACHE


def _prep_weights(w1_l0, w1_l1, w1_l2, w2_l0, w2_l1, w2_l2):
    def t(w, scale, kt):
        w = np.asarray(w, np.float32) / scale
        if kt == 1:
            return np.ascontiguousarray(w.astype(BF))
        return np.ascontiguousarray(
            w.reshape(kt, 128, w.shape[1]).transpose(1, 0, 2).astype(BF)
        )

    return {
        "w1l0": t(w1_l0, math.sqrt(512.0), 4),
        "w1l1": t(w1_l1, math.sqrt(256.0), 2),
        "w1l2": t(w1_l2, math.sqrt(128.0), 1),
        "w2l0": t(w2_l0, math.sqrt(1024.0), 8),
        "w2l1": t(w2_l1, math.sqrt(512.0), 4),
        "w2l2": t(w2_l2, math.sqrt(256.0), 2),
    }


def _prep_x(x):
    """[32768, 1920] f32 -> [8, 128, 15, 4096] bf16, feature-major tiles."""
    xc = np.asarray(x, np.float32).reshape(N_CORES, N_CORE, 1920)
    xt = np.empty((N_CORES, 128, 15, N_CORE), dtype=BF)
    l0 = xc[:, :, 0:512].reshape(N_CORES, N_CORE, 4, 128)
    xt[:, :, 0:4, :] = l0.transpose(0, 3, 2, 1).astype(BF)
    l1 = xc[:, :, 512:1280].reshape(N_CORES, N_CORE, 2, 128, 3)
    xt[:, :, 4:10, :] = (
        l1.transpose(0, 3, 2, 4, 1).reshape(N_CORES, 128, 6, N_CORE).astype(BF)
    )
    l2 = xc[:, :, 1280:1920].reshape(N_CORES, N_CORE, 128, 5)
    xt[:, :, 10:15, :] = l2.transpose(0, 2, 3, 1).astype(BF)
    return xt


def _make_in_maps(x, w1_l0, w1_l1, w1_l2, w2_l0, w2_l1, w2_l2):
    ws = _prep_weights(w1_l0, w1_l1, w1_l2, w2_l0, w2_l1, w2_l2)
    xt = _prep_x(x)
    return [
        {"xt": np.ascontiguousarray(xt[c]), **ws} for c in range(N_CORES)
    ]


def _postprocess(results):
    """per-core out [128, 15, 4096] bf16 -> [32768, 1920] f32."""
    o = np.stack([np.asarray(results[c]["out"]) for c in range(N_CORES)])
    o = o.astype(np.float32)
    out = np.empty((N_CORES, N_CORE, 1920), np.float32)
    out[:, :, 0:512] = (
        o[:, :, 0:4, :].transpose(0, 3, 2, 1).reshape(N_CORES, N_CORE, 512)
    )
    out[:, :, 512:1280] = (
        o[:, :, 4:10, :]
        .reshape(N_CORES, 128, 2, 3, N_CORE)
        .transpose(0, 4, 2, 1, 3)
        .reshape(N_CORES, N_CORE, 768)
    )
    out[:, :, 1280:1920] = (
        o[:, :, 10:15, :].transpose(0, 3, 1, 2).reshape(N_CORES, N_CORE, 640)
    )
    return np.ascontiguousarray(out.reshape(N_TOTAL, 1920))


def kernel(x, w1_l0, w1_l1, w1_l2, w2_l0, w2_l1, w2_l2):
    nc = _get_nc()
    in_maps = _make_in_maps(x, w1_l0, w1_l1, w1_l2, w2_l0, w2_l1, w2_l2)
    res = run_bass_kernel_spmd(nc, in_maps, list(range(N_CORES))).results
    return _postprocess(res)


# revision 4
# speedup vs baseline: 1.8135x; 1.0125x over previous
"""Trainium2 Bass kernel for the e3nn-style InterModule:
   out = Linear2( NormAct( Linear1(x) ) )  over irreps
     IN  [(512,0),(256,1),(128,2)]  dim 1920
     MID [(1024,0),(512,1),(256,2)] dim 3840
     OUT = IN
   N = 32768 nodes, data-parallel over 8 cores (4096 nodes/core).

v3 design (feature-major end-to-end, bf16):
  - Host pre-transposes x into de-interleaved feature-major tiles
    xt[128p, 15ft, 4096n] (bf16) and pre-transposes/prescales weights,
    so the device does NO transposes at all.
  - Linear1: stationary W1 tile [u,128v], moving xt [u, n] -> h^T in PSUM.
    Slice order: l1, l2 first; l0 last — so the l1/l2 norm chain
    (squares/adds/sqrt) runs while the l0 matmuls still stream.
  - NormAct: h copied PSUM->SBUF bf16 (ACT), squares + adds + scale-muls
    on DVE bf16 2x, one batched Sqrt + Sigmoids per block.
  - Linear2: stationary W2 tile [v,128w], moving g -> out^T in PSUM,
    emitted l1, l2 first, l0 last (l0 needs the longest scale chain);
    copied to SBUF bf16 (mostly DVE), DMA'd out feature-major; host
    re-interleaves and upcasts to f32.

hsb row map (mid irreps): l1: rows 0..12 (kv*3+j), l2: rows 12..22
(kv*5+j), l0: rows 22..30 (kv).  out^T ft map (dram): l0: 0..4 (wt),
l1: 4..10 (ko*3+j), l2: 10..15 (j).
"""

import math
from contextlib import ExitStack

import numpy as np
import ml_dtypes

import concourse.bass as bass
import concourse.tile as tile
from concourse import bacc, mybir
from concourse.bass_utils import run_bass_kernel_spmd

F32 = mybir.dt.float32
BF16 = mybir.dt.bfloat16
AF = mybir.ActivationFunctionType
ALU = mybir.AluOpType

BF = ml_dtypes.bfloat16

N_CORES = 8
N_TOTAL = 32768
N_CORE = N_TOTAL // N_CORES          # 4096
BLK = 512
NBLK = N_CORE // BLK                 # 8

# stage-3 out slices copied via ACT (rest via DVE)
S3_ACT = {0, 1, 2}
# stage-3 emission order: l1, l2 first; l0 last
S3_ORDER = list(range(4, 15)) + list(range(0, 4))


def _build():
    nc = bacc.Bacc(
        "TRN2", target_bir_lowering=False, debug=False, num_devices=N_CORES
    )

    xt_d = nc.dram_tensor("xt", [128, 15, N_CORE], BF16, kind="ExternalInput").ap()
    w1l0_d = nc.dram_tensor("w1l0", [128, 4, 1024], BF16, kind="ExternalInput").ap()
    w1l1_d = nc.dram_tensor("w1l1", [128, 2, 512], BF16, kind="ExternalInput").ap()
    w1l2_d = nc.dram_tensor("w1l2", [128, 256], BF16, kind="ExternalInput").ap()
    w2l0_d = nc.dram_tensor("w2l0", [128, 8, 512], BF16, kind="ExternalInput").ap()
    w2l1_d = nc.dram_tensor("w2l1", [128, 4, 256], BF16, kind="ExternalInput").ap()
    w2l2_d = nc.dram_tensor("w2l2", [128, 2, 128], BF16, kind="ExternalInput").ap()
    out_d = nc.dram_tensor("out", [128, 15, N_CORE], BF16, kind="ExternalOutput").ap()

    with tile.TileContext(nc) as tc, ExitStack() as ctx:
        consts = ctx.enter_context(tc.tile_pool(name="consts", bufs=1))
        sb = ctx.enter_context(tc.tile_pool(name="sb", bufs=1))
        ps = ctx.enter_context(tc.tile_pool(name="ps", bufs=1, space="PSUM"))

        w1l0 = consts.tile([128, 4, 1024], BF16)
        w1l1 = consts.tile([128, 2, 512], BF16)
        w1l2 = consts.tile([128, 256], BF16)
        w2l0 = consts.tile([128, 8, 512], BF16)
        w2l1 = consts.tile([128, 4, 256], BF16)
        w2l2 = consts.tile([128, 2, 128], BF16)
        nc.sync.dma_start(out=w1l0, in_=w1l0_d)
        nc.sync.dma_start(out=w1l1, in_=w1l1_d)
        nc.sync.dma_start(out=w1l2, in_=w1l2_d)
        nc.sync.dma_start(out=w2l0, in_=w2l0_d)
        nc.sync.dma_start(out=w2l1, in_=w2l1_d)
        nc.sync.dma_start(out=w2l2, in_=w2l2_d)

        def l1_slice(s, xt):
            """Accumulation list for mid slice s (hsb row s)."""
            if s < 12:
                kv, j = divmod(s, 3)
                return [
                    (w1l1[:, ki, kv * 128 : (kv + 1) * 128], xt[:, 4 + ki * 3 + j, :])
                    for ki in range(2)
                ]
            if s < 22:
                kv, j = divmod(s - 12, 5)
                return [(w1l2[:, kv * 128 : (kv + 1) * 128], xt[:, 10 + j, :])]
            kv = s - 22
            return [
                (w1l0[:, ki, kv * 128 : (kv + 1) * 128], xt[:, ki, :])
                for ki in range(4)
            ]

        def s3_slice(ft, hsb):
            if ft < 4:
                wt = ft
                return [
                    (w2l0[:, kv, wt * 128 : (wt + 1) * 128], hsb[:, 22 + kv, :])
                    for kv in range(8)
                ]
            if ft < 10:
                ko, j = divmod(ft - 4, 3)
                return [
                    (w2l1[:, kv, ko * 128 : (ko + 1) * 128], hsb[:, kv * 3 + j, :])
                    for kv in range(4)
                ]
            j = ft - 10
            return [(w2l2[:, kv, :], hsb[:, 12 + kv * 5 + j, :]) for kv in range(2)]

        for b in range(NBLK):
            xt = sb.tile([128, 15, BLK], BF16, name="xt", tag="xt", bufs=2)
            nc.sync.dma_start(out=xt, in_=xt_d[:, :, b * BLK : (b + 1) * BLK])

            hsb = sb.tile([128, 30, BLK], BF16, name="hsb", tag="hsb", bufs=2)
            sqb = sb.tile([128, 22, BLK], BF16, name="sqb", tag="sqb", bufs=1)
            nrm = sb.tile([128, 6, BLK], BF16, name="nrm", tag="nrm", bufs=2)
            absb = sb.tile([128, 8, BLK], BF16, name="absb", tag="absb", bufs=2)
            negb = sb.tile([128, 8, BLK], BF16, name="negb", tag="negb", bufs=1)
            outsb = sb.tile([128, 15, BLK], BF16, name="outsb", tag="outsb", bufs=2)

            # ---------------- Linear1: 30 mid slices via [3,512] PSUM tiles
            sq_done = 0
            for t in range(10):
                hm = ps.tile([128, 3, BLK], F32, name="hm", tag="hA", bufs=2)
                for r in range(3):
                    mms = l1_slice(t * 3 + r, xt)
                    for k, (w_ap, x_ap) in enumerate(mms):
                        nc.tensor.matmul(
                            hm[:, r, :], w_ap, x_ap,
                            start=(k == 0), stop=(k == len(mms) - 1),
                        )
                nc.scalar.activation(
                    out=hsb[:, t * 3 : (t + 1) * 3, :], in_=hm, func=AF.Copy
                )
                # squares (l1/l2 rows only, i.e. rows < 22) as rows land
                avail = min((t + 1) * 3, 22)
                if avail - sq_done >= 6 or (avail == 22 and avail > sq_done):
                    nc.vector.tensor_mul(
                        sqb[:, sq_done:avail, :],
                        hsb[:, sq_done:avail, :],
                        hsb[:, sq_done:avail, :],
                    )
                    sq_done = avail

            # ---------------- NormAct
            # nsq: l1 (j=3) -> nrm[0:4], l2 (j=5) -> nrm[4:6]
            v3 = sqb[:, 0:12, :].rearrange("p (k j) n -> p k j n", j=3)
            nc.vector.tensor_add(nrm[:, 0:4, :], v3[:, :, 0, :], v3[:, :, 1, :])
            nc.vector.tensor_add(nrm[:, 0:4, :], nrm[:, 0:4, :], v3[:, :, 2, :])
            v5 = sqb[:, 12:22, :].rearrange("p (k j) n -> p k j n", j=5)
            nc.vector.tensor_add(nrm[:, 4:6, :], v5[:, :, 0, :], v5[:, :, 1, :])
            for j in (2, 3, 4):
                nc.vector.tensor_add(nrm[:, 4:6, :], nrm[:, 4:6, :], v5[:, :, j, :])
            # n = sqrt(nsq)   (sqrt table set)
            nc.scalar.activation(out=nrm, in_=nrm, func=AF.Sqrt)
            # |h| for l0: max(-h, h)
            nc.vector.tensor_scalar_mul(negb, hsb[:, 22:30, :], -1.0)
            nc.vector.tensor_tensor(
                out=absb, in0=hsb[:, 22:30, :], in1=negb, op=ALU.max
            )
            # scales = sigmoid(.)  (sigmoid table set)
            nc.scalar.activation(out=nrm, in_=nrm, func=AF.Sigmoid)
            nc.scalar.activation(out=absb, in_=absb, func=AF.Sigmoid)
            # g = h * scale (in place on hsb); l1 first (feeds stage-3 head)
            h3 = hsb[:, 0:12, :].rearrange("p (k j) n -> p k j n", j=3)
            nc.vector.tensor_mul(
                h3, h3,
                nrm[:, 0:4, :].unsqueeze(2).broadcast_to([128, 4, 3, BLK]),
            )
            h5 = hsb[:, 12:22, :].rearrange("p (k j) n -> p k j n", j=5)
            nc.vector.tensor_mul(
                h5, h5,
                nrm[:, 4:6, :].unsqueeze(2).broadcast_to([128, 2, 5, BLK]),
            )
            nc.vector.tensor_mul(hsb[:, 22:30, :], hsb[:, 22:30, :], absb)

            # ---------------- Linear2: 15 out slices via [512] PSUM tiles
            for ft in S3_ORDER:
                q = ps.tile([128, BLK], F32, name="q", tag="hB", bufs=2)
                mms = s3_slice(ft, hsb)
                for k, (w_ap, g_ap) in enumerate(mms):
                    nc.tensor.matmul(
                        q, w_ap, g_ap, start=(k == 0), stop=(k == len(mms) - 1)
                    )
                if ft in S3_ACT:
                    nc.scalar.activation(out=outsb[:, ft, :], in_=q, func=AF.Copy)
                else:
                    nc.vector.tensor_copy(out=outsb[:, ft, :], in_=q)

            nc.sync.dma_start(
                out=out_d[:, :, b * BLK : (b + 1) * BLK], in_=outsb
            )

    nc.compile()
    return nc


_NC_CACHE = None


def _get_nc():
    global _NC_CACHE
    if _NC_CACHE is None:
        _NC_CACHE = _build()
    return _NC_CACHE


def _prep_weights(w1_l0, w1_l1, w1_l2, w2_l0, w2_l1, w2_l2):
    def t(w, scale, kt):
        w = np.asarray(w, np.float32) / scale
        if kt == 1:
            return np.ascontiguousarray(w.astype(BF))
        return np.ascontiguousarray(
            w.reshape(kt, 128, w.shape[1]).transpose(1, 0, 2).astype(BF)
        )

    return {
        "w1l0": t(w1_l0, math.sqrt(512.0), 4),
        "w1l1": t(w1_l1, math.sqrt(256.0), 2),
        "w1l2": t(w1_l2, math.sqrt(128.0), 1),
        "w2l0": t(w2_l0, math.sqrt(1024.0), 8),
        "w2l1": t(w2_l1, math.sqrt(512.0), 4),
        "w2l2": t(w2_l2, math.sqrt(256.0), 2),
    }


def _prep_x(x):
    """[32768, 1920] f32 -> [8, 128, 15, 4096] bf16, feature-major tiles."""
    xc = np.asarray(x, np.float32).reshape(N_CORES, N_CORE, 1920)
    xt = np.empty((N_CORES, 128, 15, N_CORE), dtype=BF)
    l0 = xc[:, :, 0:512].reshape(N_CORES, N_CORE, 4, 128)
    xt[:, :, 0:4, :] = l0.transpose(0, 3, 2, 1).astype(BF)
    l1 = xc[:, :, 512:1280].reshape(N_CORES, N_CORE, 2, 128, 3)
    xt[:, :, 4:10, :] = (
        l1.transpose(0, 3, 2, 4, 1).reshape(N_CORES, 128, 6, N_CORE).astype(BF)
    )
    l2 = xc[:, :, 1280:1920].reshape(N_CORES, N_CORE, 128, 5)
    xt[:, :, 10:15, :] = l2.transpose(0, 2, 3, 1).astype(BF)
    return xt


def _make_in_maps(x, w1_l0, w1_l1, w1_l2, w2_l0, w2_l1, w2_l2):
    ws = _prep_weights(w1_l0, w1_l1, w1_l2, w2_l0, w2_l1, w2_l2)
    xt = _prep_x(x)
    return [
        {"xt": np.ascontiguousarray(xt[c]), **ws} for c in range(N_CORES)
    ]


def _postprocess(results):
    """per-core out [128, 15, 4096] bf16 -> [32768, 1920] f32."""
    o = np.stack([np.asarray(results[c]["out"]) for c in range(N_CORES)])
    o = o.astype(np.float32)
    out = np.empty((N_CORES, N_CORE, 1920), np.float32)
    out[:, :, 0:512] = (
        o[:, :, 0:4, :].transpose(0, 3, 2, 1).reshape(N_CORES, N_CORE, 512)
    )
    out[:, :, 512:1280] = (
        o[:, :, 4:10, :]
        .reshape(N_CORES, 128, 2, 3, N_CORE)
        .transpose(0, 4, 2, 1, 3)
        .reshape(N_CORES, N_CORE, 768)
    )
    out[:, :, 1280:1920] = (
        o[:, :, 10:15, :].transpose(0, 3, 1, 2).reshape(N_CORES, N_CORE, 640)
    )
    return np.ascontiguousarray(out.reshape(N_TOTAL, 1920))


def kernel(x, w1_l0, w1_l1, w1_l2, w2_l0, w2_l1, w2_l2):
    nc = _get_nc()
    in_maps = _make_in_maps(x, w1_l0, w1_l1, w1_l2, w2_l0, w2_l1, w2_l2)
    res = run_bass_kernel_spmd(nc, in_maps, list(range(N_CORES))).results
    return _postprocess(res)


# revision 5
# speedup vs baseline: 1.9800x; 1.0918x over previous
"""Trainium2 Bass kernel for the e3nn-style InterModule:
   out = Linear2( NormAct( Linear1(x) ) )  over irreps
     IN  [(512,0),(256,1),(128,2)]  dim 1920
     MID [(1024,0),(512,1),(256,2)] dim 3840
     OUT = IN
   N = 32768 nodes, data-parallel over 8 cores (4096 nodes/core).

v3 design (feature-major end-to-end, bf16):
  - Host pre-transposes x into de-interleaved feature-major tiles
    xt[128p, 15ft, 4096n] (bf16) and pre-transposes/prescales weights,
    so the device does NO transposes at all.
  - Linear1: stationary W1 tile [u,128v], moving xt [u, n] -> h^T in PSUM.
    Slice order: l1, l2 first; l0 last — so the l1/l2 norm chain
    (squares/adds/sqrt) runs while the l0 matmuls still stream.
  - NormAct: h copied PSUM->SBUF bf16 (ACT), squares + adds + scale-muls
    on DVE bf16 2x, one batched Sqrt + Sigmoids per block.
  - Linear2: stationary W2 tile [v,128w], moving g -> out^T in PSUM,
    emitted l1, l2 first, l0 last (l0 needs the longest scale chain);
    copied to SBUF bf16 (mostly DVE), DMA'd out feature-major; host
    re-interleaves and upcasts to f32.

hsb row map (mid irreps): l1: rows 0..12 (kv*3+j), l2: rows 12..22
(kv*5+j), l0: rows 22..30 (kv).  out^T ft map (dram): l0: 0..4 (wt),
l1: 4..10 (ko*3+j), l2: 10..15 (j).
"""

import math
from contextlib import ExitStack

import numpy as np
import ml_dtypes

import concourse.bass as bass
import concourse.tile as tile
from concourse import bacc, mybir
from concourse.bass_utils import run_bass_kernel_spmd

F32 = mybir.dt.float32
BF16 = mybir.dt.bfloat16
AF = mybir.ActivationFunctionType
ALU = mybir.AluOpType

BF = ml_dtypes.bfloat16

N_CORES = 8
N_TOTAL = 32768
N_CORE = N_TOTAL // N_CORES          # 4096
BLK = 512
NBLK = N_CORE // BLK                 # 8

# stage-3 out slices copied via ACT (rest via DVE)
S3_ACT = {0, 1, 2}
# stage-3 emission order: l1, l2 first; l0 last
S3_ORDER = list(range(4, 15)) + list(range(0, 4))


def _build():
    nc = bacc.Bacc(
        "TRN2", target_bir_lowering=False, debug=False, num_devices=N_CORES
    )

    xt_d = nc.dram_tensor("xt", [128, 15, N_CORE], BF16, kind="ExternalInput").ap()
    w1l0_d = nc.dram_tensor("w1l0", [128, 4, 1024], BF16, kind="ExternalInput").ap()
    w1l1_d = nc.dram_tensor("w1l1", [128, 2, 512], BF16, kind="ExternalInput").ap()
    w1l2_d = nc.dram_tensor("w1l2", [128, 256], BF16, kind="ExternalInput").ap()
    w2l0_d = nc.dram_tensor("w2l0", [128, 8, 512], BF16, kind="ExternalInput").ap()
    w2l1_d = nc.dram_tensor("w2l1", [128, 4, 256], BF16, kind="ExternalInput").ap()
    w2l2_d = nc.dram_tensor("w2l2", [128, 2, 128], BF16, kind="ExternalInput").ap()
    out_d = nc.dram_tensor("out", [128, 15, N_CORE], BF16, kind="ExternalOutput").ap()

    with tile.TileContext(nc) as tc, ExitStack() as ctx:
        consts = ctx.enter_context(tc.tile_pool(name="consts", bufs=1))
        sb = ctx.enter_context(tc.tile_pool(name="sb", bufs=1))
        ps = ctx.enter_context(tc.tile_pool(name="ps", bufs=1, space="PSUM"))

        w1l0 = consts.tile([128, 4, 1024], BF16)
        w1l1 = consts.tile([128, 2, 512], BF16)
        w1l2 = consts.tile([128, 256], BF16)
        w2l0 = consts.tile([128, 8, 512], BF16)
        w2l1 = consts.tile([128, 4, 256], BF16)
        w2l2 = consts.tile([128, 2, 128], BF16)
        nc.sync.dma_start(out=w1l0, in_=w1l0_d)
        nc.sync.dma_start(out=w1l1, in_=w1l1_d)
        nc.sync.dma_start(out=w1l2, in_=w1l2_d)
        nc.sync.dma_start(out=w2l0, in_=w2l0_d)
        nc.sync.dma_start(out=w2l1, in_=w2l1_d)
        nc.sync.dma_start(out=w2l2, in_=w2l2_d)

        def l1_slice(s, xt):
            """Accumulation list for mid slice s (hsb row s)."""
            if s < 12:
                kv, j = divmod(s, 3)
                return [
                    (w1l1[:, ki, kv * 128 : (kv + 1) * 128], xt[:, 4 + ki * 3 + j, :])
                    for ki in range(2)
                ]
            if s < 22:
                kv, j = divmod(s - 12, 5)
                return [(w1l2[:, kv * 128 : (kv + 1) * 128], xt[:, 10 + j, :])]
            kv = s - 22
            return [
                (w1l0[:, ki, kv * 128 : (kv + 1) * 128], xt[:, ki, :])
                for ki in range(4)
            ]

        def s3_slice(ft, hsb):
            if ft < 4:
                wt = ft
                return [
                    (w2l0[:, kv, wt * 128 : (wt + 1) * 128], hsb[:, 22 + kv, :])
                    for kv in range(8)
                ]
            if ft < 10:
                ko, j = divmod(ft - 4, 3)
                return [
                    (w2l1[:, kv, ko * 128 : (ko + 1) * 128], hsb[:, kv * 3 + j, :])
                    for kv in range(4)
                ]
            j = ft - 10
            return [(w2l2[:, kv, :], hsb[:, 12 + kv * 5 + j, :]) for kv in range(2)]

        def emit_l1(b):
            """DMA xt(b), Linear1 MMs + PSUM->SBUF copies + squares.
            Returns the per-block tile state."""
            st = {}
            st["xt"] = xt = sb.tile([128, 15, BLK], BF16, name="xt", tag="xt", bufs=2)
            nc.sync.dma_start(out=xt, in_=xt_d[:, :, b * BLK : (b + 1) * BLK])
            st["hsb"] = hsb = sb.tile(
                [128, 30, BLK], BF16, name="hsb", tag="hsb", bufs=2
            )
            st["sqb"] = sqb = sb.tile(
                [128, 22, BLK], BF16, name="sqb", tag="sqb", bufs=1
            )
            st["nrm"] = sb.tile([128, 6, BLK], BF16, name="nrm", tag="nrm", bufs=2)
            st["absb"] = sb.tile([128, 8, BLK], BF16, name="absb", tag="absb", bufs=2)
            st["negb"] = sb.tile([128, 8, BLK], BF16, name="negb", tag="negb", bufs=1)
            st["outsb"] = sb.tile(
                [128, 15, BLK], BF16, name="outsb", tag="outsb", bufs=2
            )
            sq_done = 0
            for t in range(10):
                hm = ps.tile([128, 3, BLK], F32, name="hm", tag="hA", bufs=2)
                for r in range(3):
                    mms = l1_slice(t * 3 + r, xt)
                    for k, (w_ap, x_ap) in enumerate(mms):
                        nc.tensor.matmul(
                            hm[:, r, :], w_ap, x_ap,
                            start=(k == 0), stop=(k == len(mms) - 1),
                        )
                nc.scalar.activation(
                    out=hsb[:, t * 3 : (t + 1) * 3, :], in_=hm, func=AF.Copy
                )
                # squares (l1/l2 rows only, i.e. rows < 22) as rows land
                avail = min((t + 1) * 3, 22)
                if avail - sq_done >= 6 or (avail == 22 and avail > sq_done):
                    nc.vector.tensor_mul(
                        sqb[:, sq_done:avail, :],
                        hsb[:, sq_done:avail, :],
                        hsb[:, sq_done:avail, :],
                    )
                    sq_done = avail
            return st

        def emit_normact(st):
            hsb, sqb, nrm = st["hsb"], st["sqb"], st["nrm"]
            absb, negb = st["absb"], st["negb"]
            # nsq: l1 (j=3) -> nrm[0:4], l2 (j=5) -> nrm[4:6]
            v3 = sqb[:, 0:12, :].rearrange("p (k j) n -> p k j n", j=3)
            nc.vector.tensor_add(nrm[:, 0:4, :], v3[:, :, 0, :], v3[:, :, 1, :])
            nc.vector.tensor_add(nrm[:, 0:4, :], nrm[:, 0:4, :], v3[:, :, 2, :])
            v5 = sqb[:, 12:22, :].rearrange("p (k j) n -> p k j n", j=5)
            nc.vector.tensor_add(nrm[:, 4:6, :], v5[:, :, 0, :], v5[:, :, 1, :])
            for j in (2, 3, 4):
                nc.vector.tensor_add(nrm[:, 4:6, :], nrm[:, 4:6, :], v5[:, :, j, :])
            # n = sqrt(nsq)   (sqrt table set)
            nc.scalar.activation(out=nrm, in_=nrm, func=AF.Sqrt)
            # |h| for l0: max(-h, h)
            nc.vector.tensor_scalar_mul(negb, hsb[:, 22:30, :], -1.0)
            nc.vector.tensor_tensor(
                out=absb, in0=hsb[:, 22:30, :], in1=negb, op=ALU.max
            )
            # scales = sigmoid(.)  (sigmoid table set)
            nc.scalar.activation(out=nrm, in_=nrm, func=AF.Sigmoid)
            nc.scalar.activation(out=absb, in_=absb, func=AF.Sigmoid)
            # g = h * scale (in place on hsb); l1 first (feeds stage-3 head)
            h3 = hsb[:, 0:12, :].rearrange("p (k j) n -> p k j n", j=3)
            nc.vector.tensor_mul(
                h3, h3,
                nrm[:, 0:4, :].unsqueeze(2).broadcast_to([128, 4, 3, BLK]),
            )
            h5 = hsb[:, 12:22, :].rearrange("p (k j) n -> p k j n", j=5)
            nc.vector.tensor_mul(
                h5, h5,
                nrm[:, 4:6, :].unsqueeze(2).broadcast_to([128, 2, 5, BLK]),
            )
            nc.vector.tensor_mul(hsb[:, 22:30, :], hsb[:, 22:30, :], absb)

        def emit_s3(b, st):
            hsb, outsb = st["hsb"], st["outsb"]
            for ft in S3_ORDER:
                q = ps.tile([128, BLK], F32, name="q", tag="hB", bufs=2)
                mms = s3_slice(ft, hsb)
                for k, (w_ap, g_ap) in enumerate(mms):
                    nc.tensor.matmul(
                        q, w_ap, g_ap, start=(k == 0), stop=(k == len(mms) - 1)
                    )
                if ft in S3_ACT:
                    nc.scalar.activation(out=outsb[:, ft, :], in_=q, func=AF.Copy)
                else:
                    nc.vector.tensor_copy(out=outsb[:, ft, :], in_=q)
            nc.sync.dma_start(
                out=out_d[:, :, b * BLK : (b + 1) * BLK], in_=outsb
            )

        # software-pipelined emission: PE always has L1(b+1) queued while
        # block b's norm chain completes, then drains s3(b).
        st = emit_l1(0)
        for b in range(NBLK):
            emit_normact(st)
            nxt = emit_l1(b + 1) if b + 1 < NBLK else None
            emit_s3(b, st)
            st = nxt

    nc.compile()
    return nc


_NC_CACHE = None


def _get_nc():
    global _NC_CACHE
    if _NC_CACHE is None:
        _NC_CACHE = _build()
    return _NC_CACHE


def _prep_weights(w1_l0, w1_l1, w1_l2, w2_l0, w2_l1, w2_l2):
    def t(w, scale, kt):
        w = np.asarray(w, np.float32) / scale
        if kt == 1:
            return np.ascontiguousarray(w.astype(BF))
        return np.ascontiguousarray(
            w.reshape(kt, 128, w.shape[1]).transpose(1, 0, 2).astype(BF)
        )

    return {
        "w1l0": t(w1_l0, math.sqrt(512.0), 4),
        "w1l1": t(w1_l1, math.sqrt(256.0), 2),
        "w1l2": t(w1_l2, math.sqrt(128.0), 1),
        "w2l0": t(w2_l0, math.sqrt(1024.0), 8),
        "w2l1": t(w2_l1, math.sqrt(512.0), 4),
        "w2l2": t(w2_l2, math.sqrt(256.0), 2),
    }


def _prep_x(x):
    """[32768, 1920] f32 -> [8, 128, 15, 4096] bf16, feature-major tiles."""
    xc = np.asarray(x, np.float32).reshape(N_CORES, N_CORE, 1920)
    xt = np.empty((N_CORES, 128, 15, N_CORE), dtype=BF)
    l0 = xc[:, :, 0:512].reshape(N_CORES, N_CORE, 4, 128)
    xt[:, :, 0:4, :] = l0.transpose(0, 3, 2, 1).astype(BF)
    l1 = xc[:, :, 512:1280].reshape(N_CORES, N_CORE, 2, 128, 3)
    xt[:, :, 4:10, :] = (
        l1.transpose(0, 3, 2, 4, 1).reshape(N_CORES, 128, 6, N_CORE).astype(BF)
    )
    l2 = xc[:, :, 1280:1920].reshape(N_CORES, N_CORE, 128, 5)
    xt[:, :, 10:15, :] = l2.transpose(0, 2, 3, 1).astype(BF)
    return xt


def _make_in_maps(x, w1_l0, w1_l1, w1_l2, w2_l0, w2_l1, w2_l2):
    ws = _prep_weights(w1_l0, w1_l1, w1_l2, w2_l0, w2_l1, w2_l2)
    xt = _prep_x(x)
    return [
        {"xt": np.ascontiguousarray(xt[c]), **ws} for c in range(N_CORES)
    ]


def _postprocess(results):
    """per-core out [128, 15, 4096] bf16 -> [32768, 1920] f32."""
    o = np.stack([np.asarray(results[c]["out"]) for c in range(N_CORES)])
    o = o.astype(np.float32)
    out = np.empty((N_CORES, N_CORE, 1920), np.float32)
    out[:, :, 0:512] = (
        o[:, :, 0:4, :].transpose(0, 3, 2, 1).reshape(N_CORES, N_CORE, 512)
    )
    out[:, :, 512:1280] = (
        o[:, :, 4:10, :]
        .reshape(N_CORES, 128, 2, 3, N_CORE)
        .transpose(0, 4, 2, 1, 3)
        .reshape(N_CORES, N_CORE, 768)
    )
    out[:, :, 1280:1920] = (
        o[:, :, 10:15, :].transpose(0, 3, 1, 2).reshape(N_CORES, N_CORE, 640)
    )
    return np.ascontiguousarray(out.reshape(N_TOTAL, 1920))


def kernel(x, w1_l0, w1_l1, w1_l2, w2_l0, w2_l1, w2_l2):
    nc = _get_nc()
    in_maps = _make_in_maps(x, w1_l0, w1_l1, w1_l2, w2_l0, w2_l1, w2_l2)
    res = run_bass_kernel_spmd(nc, in_maps, list(range(N_CORES))).results
    return _postprocess(res)
